# revision 5
# baseline (speedup 1.0000x reference)
"""GCNConv kernel for 8 Trainium2 NeuronCores.

Math: out = CSR_neighbor_sum(X @ W) == (CSR_neighbor_sum(X)) @ W
(the unweighted neighbor sum commutes with the right-multiplication by W),
so each core gathers+sums raw X rows and applies the small [128,128] weight
matmul afterwards.

Strategy (hardcoded for N=100000 nodes, degree<=16, D=128, 8 cores):
  - Output nodes sharded across 8 cores (12500 rows each); X, W replicated.
  - Neighbor rows are fetched with the batched SWDGE `dma_gather` custom
    instruction (bf16 table, 256B rows, int16 indices) spread across all
    4 SWDGE queues (4 Q7 core pairs run concurrently), which measures ~1ns
    per gathered row vs ~6ns/row for the classic one-row-per-partition
    indirect-DMA path.
  - int16 indices only address 32K rows, so X is split into 4 windows of
    25088 rows (a zero row is appended to each window for padding). Each
    128-node tile issues one dma_gather per window; slots are node-major
    ([128 partitions=nodes, M chunks]) with a per-tile compile-time width
    M[t][w] = max window-count over the tile's nodes (and over cores, to
    keep the program SPMD). Nodes are sorted per core by their window-count
    vector so tiles are homogeneous (total pad ~1.4x edges instead of the
    ~2.2x of unsorted max-padding); the host un-permutes the output rows.
  - A strided DVE binary tree sums each window tile's chunks (bf16
    intermediate, fp32 combine), PE transposes the sum and multiplies by W
    (fp32), and the result is stored to the core's output shard.

The NEFF is compiled per edge-structure signature (the M widths); for a
given graph the kernel compiles once and is cached.
"""

import time

import numpy as np

N_NODES = 100000
DEG = 16
D = 128
N_CORES = 8
NODES_PER_CORE = N_NODES // N_CORES  # 12500
P = 128  # SBUF partitions / nodes per tile
N_TILES = (NODES_PER_CORE + P - 1) // P  # 98
SENTINEL = N_NODES  # "no edge" marker in the host edge matrix
NW = 4  # index windows (int16 limit)
WIN = 25088  # rows per window (4 * 25088 >= 100000)
WROWS = WIN + 1  # +1 zero pad row per window in the gather table

_CACHE = {}


def _build_nc(m_widths):
    """Construct and compile the SPMD per-core Bass program. `m_widths` is
    an [N_TILES][NW] tuple of per-tile per-window chunk counts."""
    import concourse.mybir as mybir
    from concourse import bacc
    from concourse.tile import TileContext
    from concourse.masks import make_identity

    tail = NODES_PER_CORE - (N_TILES - 1) * P  # 84
    s_tiles = [sum(mw) for mw in m_widths]  # chunks per tile
    idx_cols = 8 * sum(s_tiles)  # int16 idx cols per partition

    nc = bacc.Bacc("TRN2", target_bir_lowering=False, debug=False,
                   enable_asserts=True, num_devices=N_CORES,
                   dynamic_dma_scratch_size=65536,
                   num_swdge_queues=NW)
    XT = nc.dram_tensor("XT", [NW * WROWS, D], mybir.dt.bfloat16,
                        kind="ExternalInput")
    W = nc.dram_tensor("W", [D, D], mybir.dt.float32, kind="ExternalInput")
    idx = nc.dram_tensor("idx", [P, idx_cols], mybir.dt.int16,
                         kind="ExternalInput")
    out = nc.dram_tensor("out", [NODES_PER_CORE, D], mybir.dt.float32,
                         kind="ExternalOutput")

    with TileContext(nc) as tc:
        with (
            tc.tile_pool(name="const", bufs=1) as cpool,
            tc.tile_pool(name="g0", bufs=3) as gp0,
            tc.tile_pool(name="g1", bufs=3) as gp1,
            tc.tile_pool(name="g2", bufs=3) as gp2,
            tc.tile_pool(name="g3", bufs=3) as gp3,
            tc.tile_pool(name="yp", bufs=3) as ypool,
            tc.tile_pool(name="op", bufs=3) as opool,
            tc.tile_pool(name="ps", bufs=4, space="PSUM") as pspool,
        ):
            gpools = [gp0, gp1, gp2, gp3]
            w_sb = cpool.tile([D, D], mybir.dt.float32)
            nc.sync.dma_start(out=w_sb[:], in_=W[:])
            ident = cpool.tile([P, P], mybir.dt.float32)
            make_identity(nc, ident[:])
            idx_sb = cpool.tile([P, idx_cols], mybir.dt.int16)
            nc.sync.dma_start(out=idx_sb[:], in_=idx[:])

            col = 0
            for t in range(N_TILES):
                nt = P if t < N_TILES - 1 else tail
                r0 = t * P
                gtiles = []
                for w in range(NW):
                    m = m_widths[t][w]
                    if m == 0:
                        gtiles.append(None)
                        continue
                    g = gpools[w].tile([P, m, D], mybir.dt.bfloat16,
                                       tag=f"g{w}")
                    ni = m * P
                    nc.gpsimd.dma_gather(
                        g[:, :, :],
                        XT[w * WROWS:(w + 1) * WROWS, :],
                        idx_sb[:, col:col + m * 8],
                        ni, ni, D, queue_num=w, single_packet=False)
                    col += m * 8
                    # in-window binary tree over the m chunks (bf16)
                    while m > 1:
                        h = m // 2  # adds; m-h chunks survive
                        nc.vector.tensor_add(
                            out=g[:, 0:h, :],
                            in0=g[:, 0:h, :],
                            in1=g[:, m - h:m, :],
                        )
                        m -= h
                    gtiles.append(g)
                live = [g for g in gtiles if g is not None]

                # combine the window sums in fp32
                y = ypool.tile([P, D], mybir.dt.float32, tag="y")
                if len(live) == 1:
                    nc.vector.tensor_copy(out=y[:, :], in_=live[0][:, 0, :])
                else:
                    nc.vector.tensor_add(out=y[:, :], in0=live[0][:, 0, :],
                                         in1=live[1][:, 0, :])
                    for g in live[2:]:
                        nc.vector.tensor_add(out=y[:, :], in0=y[:, :],
                                             in1=g[:, 0, :])

                # transpose Y then multiply by W: out = (Y^T)^T @ W = Y @ W
                yt_ps = pspool.tile([D, P], mybir.dt.float32, tag="yt")
                nc.tensor.transpose(out=yt_ps[:, :nt], in_=y[:nt, :],
                                    identity=ident[:nt, :nt])
                yt_sb = ypool.tile([D, P], mybir.dt.float32, tag="yt_sb")
                nc.vector.tensor_copy(out=yt_sb[:, :nt], in_=yt_ps[:, :nt])

                o_ps = pspool.tile([P, D], mybir.dt.float32, tag="ops")
                nc.tensor.matmul(out=o_ps[:nt, :], lhsT=yt_sb[:, :nt],
                                 rhs=w_sb[:], start=True, stop=True)
                o_sb = opool.tile([P, D], mybir.dt.float32, tag="osb")
                nc.vector.tensor_copy(out=o_sb[:nt, :], in_=o_ps[:nt, :])
                nc.sync.dma_start(out=out[r0:r0 + nt, :], in_=o_sb[:nt, :])
    nc.compile()
    return nc


def _get_nc(m_widths):
    key = tuple(tuple(mw) for mw in m_widths)
    if key not in _CACHE:
        _CACHE[key] = _build_nc(key)
    return _CACHE[key]


def _edge_matrix(row_pointers, column_index):
    """Per-node [N_NODES, w_pad] int32 neighbor matrix from the CSR arrays,
    padded with SENTINEL. Fast path for uniform degree DEG."""
    rp = np.asarray(row_pointers).astype(np.int64)
    ci = np.asarray(column_index).astype(np.int32)
    deg = np.diff(rp)
    if len(deg) == N_NODES and (deg == DEG).all() and rp[0] == 0 \
            and rp[-1] == len(ci):
        return ci.reshape(N_NODES, DEG), DEG
    e = np.arange(len(ci), dtype=np.int64)
    rows = np.searchsorted(rp, e, side="right") - 1
    valid = (rows >= 0) & (rows < N_NODES)
    rows = rows[valid]
    cols = ci[valid]
    order = np.argsort(rows, kind="stable")
    rows, cols = rows[order], cols[order]
    counts = np.bincount(rows, minlength=N_NODES)
    w_pad = max(int(counts.max()) if len(counts) else 1, 1)
    mat = np.full((N_NODES, w_pad), SENTINEL, dtype=np.int32)
    starts = np.zeros(N_NODES + 1, dtype=np.int64)
    np.cumsum(counts, out=starts[1:])
    slot = np.arange(len(rows)) - starts[rows]
    mat[rows, slot] = np.clip(cols, 0, N_NODES - 1)
    return mat, w_pad


def _plan(edges):
    """Host planning: per-core node ordering + per-tile window widths.

    Returns (orders [N_CORES, N_TILES*P] node ids incl. pad -1,
             m_widths [N_TILES][NW] global (max over cores) chunk counts,
             counts [N_NODES, NW])."""
    wofe = np.minimum(edges // WIN, NW - 1)  # window of each edge slot
    wofe[edges >= N_NODES] = -1  # sentinel/pad slots belong to no window
    counts = np.zeros((N_NODES, NW), np.int32)
    for w in range(NW):
        counts[:, w] = (wofe == w).sum(1)

    orders = np.full((N_CORES, N_TILES * P), -1, np.int64)
    m_per_core = np.zeros((N_CORES, N_TILES, NW), np.int32)
    for c in range(N_CORES):
        lo = c * NODES_PER_CORE
        cc = counts[lo:lo + NODES_PER_CORE]
        o = np.lexsort((cc[:, 3], cc[:, 2], cc[:, 1], cc[:, 0]))
        orders[c, :NODES_PER_CORE] = lo + o
        srt = cc[o]
        full = N_TILES - 1
        m_per_core[c, :full] = srt[:full * P].reshape(full, P, NW).max(1)
        m_per_core[c, full] = srt[full * P:].max(0)
    m_widths = m_per_core.max(0)  # SPMD: same widths on every core
    return orders, m_widths, counts


def _build_idx(edges, orders, m_widths, counts, core):
    """Per-core int16 index array [P, 8*sum(S_t)] in dma_gather layout."""
    s_tiles = m_widths.sum(1)
    idx_cols = 8 * int(s_tiles.sum())
    out = np.empty((16, idx_cols), np.int16)
    col = 0
    order = orders[core]
    wofe = np.minimum(edges // WIN, NW - 1)
    for t in range(N_TILES):
        nodes = order[t * P:(t + 1) * P]  # [-1] pads possible
        valid = nodes >= 0
        nb = edges[np.clip(nodes, 0, N_NODES - 1)]  # [P, w_pad]
        nw = wofe[np.clip(nodes, 0, N_NODES - 1)]
        nw[~valid] = -1
        nw[nb >= N_NODES] = -1  # sentinel slots
        for w in range(NW):
            m = int(m_widths[t, w])
            if m == 0:
                continue
            blk = np.full((P, m), WIN, np.int16)  # zero-pad row (local idx)
            sel = nw == w
            cnt = sel.sum(1)
            # local indices of this window's edges, left-packed per node
            for p in np.nonzero(cnt)[0]:
                vals = nb[p][sel[p]] - w * WIN
                blk[p, :len(vals)] = vals.astype(np.int16)
            # positions: i = j*128+p -> wrapped [16, ni/16]:
            # idx16[i%16, i//16] ; i//16 = j*8 + p//16
            ib = blk.T.reshape(m * P)  # i-ordered
            out[:, col:col + m * 8] = ib.reshape(m * 8, 16).T
            col += m * 8
    return np.tile(out, (8, 1))


def _make_xt(X):
    """bf16 gather table: 4 windows of WIN rows, each + 1 zero row."""
    import ml_dtypes
    xt = np.zeros((NW * WROWS, D), dtype=ml_dtypes.bfloat16)
    Xb = X.astype(ml_dtypes.bfloat16)
    for w in range(NW):
        lo = w * WIN
        hi = min(lo + WIN, N_NODES)
        if hi > lo:
            xt[w * WROWS:w * WROWS + (hi - lo)] = Xb[lo:hi]
    return xt


def kernel(X, weights, row_pointers, column_index, blockPartition,
           edgeToColumn, edgeToRow):
    from concourse.bass_utils import run_bass_kernel_spmd

    X = np.asarray(X, dtype=np.float32)
    W = np.ascontiguousarray(np.asarray(weights), dtype=np.float32)
    edges, w_pad = _edge_matrix(row_pointers, column_index)
    orders, m_widths, counts = _plan(edges)
    xt = _make_xt(X)

    nc = _get_nc(m_widths)
    in_maps = []
    for c in range(N_CORES):
        in_maps.append({
            "XT": xt,
            "W": W,
            "idx": _build_idx(edges, orders, m_widths, counts, c),
        })
    last_exc = None
    for _attempt in range(3):
        try:
            res = run_bass_kernel_spmd(nc, in_maps,
                                       core_ids=list(range(N_CORES)))
            break
        except Exception as exc:  # transient NRT/axon errors recover on retry
            last_exc = exc
            time.sleep(15)
    else:
        raise last_exc
    out = np.empty((N_NODES, D), np.float32)
    for c in range(N_CORES):
        rows = res.results[c]["out"]  # sorted-node order
        out[orders[c, :NODES_PER_CORE]] = rows
    return out


# revision 6
# speedup vs baseline: 1.0464x; 1.0464x over previous
"""GCNConv kernel for 8 Trainium2 NeuronCores.

Math: out = CSR_neighbor_sum(X @ W) == (CSR_neighbor_sum(X)) @ W
(the unweighted neighbor sum commutes with the right-multiplication by W),
so each core gathers+sums raw X rows and applies the small [128,128] weight
matmul afterwards.

Strategy (hardcoded for N=100000 nodes, degree<=16, D=128, 8 cores):
  - Output nodes sharded across 8 cores (12500 rows each); X, W replicated.
  - Neighbor rows are fetched with the batched SWDGE `dma_gather` custom
    instruction (bf16 table, 256B rows, int16 indices) spread across all
    4 SWDGE queues (4 Q7 core pairs run concurrently); measured ~1ns per
    gathered row at >=1536 indices/instruction vs ~6ns/row for the classic
    one-row-per-partition indirect-DMA path.
  - int16 indices only address 32K rows, so X is split into 4 windows of
    25088 rows (a zero row is appended to each window for padding). Tiles
    of 128 nodes are processed in PAIRS: one dma_gather per (pair, window)
    keeps indices/instruction high. Slots are node-major ([128 partitions =
    nodes, chunks]) with compile-time per-tile widths M[t][w] = max window
    count over the tile's nodes (and over cores, to stay SPMD). Nodes are
    sorted per core by window-count vector so tiles are homogeneous (total
    pad ~1.5x edges instead of ~2.2x unsorted); the host un-permutes the
    output rows at the end.
  - A strided DVE binary tree sums each tile's window chunks (bf16
    intermediates, fp32 combine), PE transposes the sum and multiplies by
    W (fp32), and the result is stored to the core's output shard.

The NEFF is compiled per edge-structure signature (the M widths); for a
given graph the kernel compiles once and is cached.
"""

import time

import numpy as np

N_NODES = 100000
DEG = 16
D = 128
N_CORES = 8
NODES_PER_CORE = N_NODES // N_CORES  # 12500
P = 128  # SBUF partitions / nodes per tile
N_TILES = (NODES_PER_CORE + P - 1) // P  # 98
SENTINEL = N_NODES  # "no edge" marker in the host edge matrix
NW = 4  # index windows (int16 limit)
WIN = 25088  # rows per window (4 * 25088 >= 100000)
WROWS = WIN + 1  # +1 zero pad row per window in the gather table
K_PAIR = 2  # node tiles per gather instruction group

_CACHE = {}


def _supers(m_widths):
    """Group tiles into consecutive groups of K_PAIR."""
    groups = []
    t = 0
    while t < N_TILES:
        groups.append(list(range(t, min(t + K_PAIR, N_TILES))))
        t += K_PAIR
    return groups


def _build_nc(m_widths):
    """Construct and compile the SPMD per-core Bass program. `m_widths` is
    an [N_TILES][NW] tuple of per-tile per-window chunk counts."""
    import concourse.mybir as mybir
    from concourse import bacc
    from concourse.tile import TileContext
    from concourse.masks import make_identity

    m_widths = np.asarray(m_widths, np.int32)
    tail = NODES_PER_CORE - (N_TILES - 1) * P  # 84
    groups = _supers(m_widths)
    # per group/window chunk counts and idx column layout
    gw_chunks = [[int(m_widths[g, w].sum()) for w in range(NW)]
                 for g in groups]
    idx_cols_per_group = [8 * sum(gw) for gw in gw_chunks]
    idx_cols = sum(idx_cols_per_group)

    nc = bacc.Bacc("TRN2", target_bir_lowering=False, debug=False,
                   enable_asserts=True, num_devices=N_CORES,
                   dynamic_dma_scratch_size=65536,
                   num_swdge_queues=NW)
    XT = nc.dram_tensor("XT", [NW * WROWS, D], mybir.dt.bfloat16,
                        kind="ExternalInput")
    W = nc.dram_tensor("W", [D, D], mybir.dt.float32, kind="ExternalInput")
    idx = nc.dram_tensor("idx", [P, idx_cols], mybir.dt.int16,
                         kind="ExternalInput")
    out = nc.dram_tensor("out", [NODES_PER_CORE, D], mybir.dt.float32,
                         kind="ExternalOutput")

    with TileContext(nc) as tc:
        with (
            tc.tile_pool(name="const", bufs=1) as cpool,
            tc.tile_pool(name="ip", bufs=3) as ipool,
            tc.tile_pool(name="g0", bufs=2) as gp0,
            tc.tile_pool(name="g1", bufs=2) as gp1,
            tc.tile_pool(name="g2", bufs=2) as gp2,
            tc.tile_pool(name="g3", bufs=2) as gp3,
            tc.tile_pool(name="yp", bufs=3) as ypool,
            tc.tile_pool(name="op", bufs=3) as opool,
            tc.tile_pool(name="ps", bufs=4, space="PSUM") as pspool,
        ):
            gpools = [gp0, gp1, gp2, gp3]
            w_sb = cpool.tile([D, D], mybir.dt.float32)
            nc.sync.dma_start(out=w_sb[:], in_=W[:])
            ident = cpool.tile([P, P], mybir.dt.float32)
            make_identity(nc, ident[:])

            col = 0
            for gi, group in enumerate(groups):
                gw = gw_chunks[gi]
                icols = idx_cols_per_group[gi]
                idx_sb = ipool.tile([P, icols], mybir.dt.int16, tag="idx")
                nc.sync.dma_start(out=idx_sb[:],
                                  in_=idx[:, col:col + icols])
                col += icols

                # one gather per window covering the whole group
                gtiles = []
                icol = 0
                for w in range(NW):
                    chunks = gw[w]
                    if chunks == 0:
                        gtiles.append(None)
                        continue
                    g = gpools[w].tile([P, chunks, D], mybir.dt.bfloat16,
                                       tag=f"g{w}")
                    ni = chunks * P
                    nc.gpsimd.dma_gather(
                        g[:, :, :],
                        XT[w * WROWS:(w + 1) * WROWS, :],
                        idx_sb[:, icol:icol + chunks * 8],
                        ni, ni, D, queue_num=w, single_packet=False)
                    icol += chunks * 8
                    gtiles.append(g)

                # per member tile: tree-sum its slice of each window tile
                for k, t in enumerate(group):
                    nt = P if t < N_TILES - 1 else tail
                    r0 = t * P
                    parts = []
                    for w in range(NW):
                        m = int(m_widths[t, w])
                        if m == 0:
                            continue
                        off = int(m_widths[group[:k], w].sum())
                        g = gtiles[w]
                        while m > 1:
                            h = m // 2
                            nc.vector.tensor_add(
                                out=g[:, off:off + h, :],
                                in0=g[:, off:off + h, :],
                                in1=g[:, off + m - h:off + m, :],
                            )
                            m -= h
                        parts.append((g, off))

                    y = ypool.tile([P, D], mybir.dt.float32, tag="y")
                    if len(parts) == 1:
                        g, off = parts[0]
                        nc.vector.tensor_copy(out=y[:, :], in_=g[:, off, :])
                    else:
                        (g0_, o0), (g1_, o1) = parts[0], parts[1]
                        nc.vector.tensor_add(out=y[:, :], in0=g0_[:, o0, :],
                                             in1=g1_[:, o1, :])
                        for g, off in parts[2:]:
                            nc.vector.tensor_add(out=y[:, :], in0=y[:, :],
                                                 in1=g[:, off, :])

                    yt_ps = pspool.tile([D, P], mybir.dt.float32, tag="yt")
                    nc.tensor.transpose(out=yt_ps[:, :nt], in_=y[:nt, :],
                                        identity=ident[:nt, :nt])
                    yt_sb = ypool.tile([D, P], mybir.dt.float32, tag="yt_sb")
                    nc.vector.tensor_copy(out=yt_sb[:, :nt],
                                          in_=yt_ps[:, :nt])

                    o_ps = pspool.tile([P, D], mybir.dt.float32, tag="ops")
                    nc.tensor.matmul(out=o_ps[:nt, :], lhsT=yt_sb[:, :nt],
                                     rhs=w_sb[:], start=True, stop=True)
                    o_sb = opool.tile([P, D], mybir.dt.float32, tag="osb")
                    nc.vector.tensor_copy(out=o_sb[:nt, :], in_=o_ps[:nt, :])
                    nc.sync.dma_start(out=out[r0:r0 + nt, :],
                                      in_=o_sb[:nt, :])
    nc.compile()
    return nc


def _get_nc(m_widths):
    key = tuple(tuple(int(v) for v in mw) for mw in m_widths)
    if key not in _CACHE:
        _CACHE[key] = _build_nc(key)
    return _CACHE[key]


def _edge_matrix(row_pointers, column_index):
    """Per-node [N_NODES, w_pad] int32 neighbor matrix from the CSR arrays,
    padded with SENTINEL. Fast path for uniform degree DEG."""
    rp = np.asarray(row_pointers).astype(np.int64)
    ci = np.asarray(column_index).astype(np.int32)
    deg = np.diff(rp)
    if len(deg) == N_NODES and (deg == DEG).all() and rp[0] == 0 \
            and rp[-1] == len(ci):
        return ci.reshape(N_NODES, DEG), DEG
    e = np.arange(len(ci), dtype=np.int64)
    rows = np.searchsorted(rp, e, side="right") - 1
    valid = (rows >= 0) & (rows < N_NODES)
    rows = rows[valid]
    cols = ci[valid]
    order = np.argsort(rows, kind="stable")
    rows, cols = rows[order], cols[order]
    counts = np.bincount(rows, minlength=N_NODES)
    w_pad = max(int(counts.max()) if len(counts) else 1, 1)
    mat = np.full((N_NODES, w_pad), SENTINEL, dtype=np.int32)
    starts = np.zeros(N_NODES + 1, dtype=np.int64)
    np.cumsum(counts, out=starts[1:])
    slot = np.arange(len(rows)) - starts[rows]
    mat[rows, slot] = np.clip(cols, 0, N_NODES - 1)
    return mat, w_pad


def _plan(edges):
    """Host planning: per-core node ordering + per-tile window widths."""
    wofe = np.minimum(edges // WIN, NW - 1)
    wofe[edges >= N_NODES] = -1
    counts = np.zeros((N_NODES, NW), np.int32)
    for w in range(NW):
        counts[:, w] = (wofe == w).sum(1)

    orders = np.full((N_CORES, N_TILES * P), -1, np.int64)
    m_per_core = np.zeros((N_CORES, N_TILES, NW), np.int32)
    for c in range(N_CORES):
        lo = c * NODES_PER_CORE
        cc = counts[lo:lo + NODES_PER_CORE]
        o = np.lexsort((cc[:, 3], cc[:, 2], cc[:, 1], cc[:, 0]))
        orders[c, :NODES_PER_CORE] = lo + o
        srt = cc[o]
        full = N_TILES - 1
        m_per_core[c, :full] = srt[:full * P].reshape(full, P, NW).max(1)
        m_per_core[c, full] = srt[full * P:].max(0)
    m_widths = m_per_core.max(0)
    return orders, m_widths, counts


def _tile_block(edges, wofe, nodes, w, m):
    """int16 index block [16, m*8] for one (tile, window)."""
    valid = nodes >= 0
    nb = edges[np.clip(nodes, 0, N_NODES - 1)]
    nw = wofe[np.clip(nodes, 0, N_NODES - 1)].copy()
    nw[~valid] = -1
    blk = np.full((P, m), WIN, np.int16)
    sel = nw == w
    cnt = sel.sum(1)
    for p in np.nonzero(cnt)[0]:
        vals = nb[p][sel[p]] - w * WIN
        blk[p, :len(vals)] = vals.astype(np.int16)
    ib = blk.T.reshape(m * P)  # position-ordered (i = j*128 + p)
    return ib.reshape(m * 8, 16).T


def _build_idx(edges, orders, m_widths, core):
    """Per-core int16 index array [P, idx_cols] in dma_gather layout,
    grouped by (tile-pair group, window, member tile)."""
    m_widths = np.asarray(m_widths, np.int32)
    groups = _supers(m_widths)
    idx_cols = 8 * int(sum(m_widths[g].sum() for g in groups))
    out = np.empty((16, idx_cols), np.int16)
    wofe = np.minimum(edges // WIN, NW - 1)
    wofe[edges >= N_NODES] = -1
    order = orders[core]
    col = 0
    for group in groups:
        for w in range(NW):
            for t in group:
                m = int(m_widths[t, w])
                if m == 0:
                    continue
                nodes = order[t * P:(t + 1) * P]
                out[:, col:col + m * 8] = _tile_block(
                    edges, wofe, nodes, w, m)
                col += m * 8
    return np.tile(out, (8, 1))


def _make_xt(X):
    """bf16 gather table: 4 windows of WIN rows, each + 1 zero row."""
    import ml_dtypes
    xt = np.zeros((NW * WROWS, D), dtype=ml_dtypes.bfloat16)
    Xb = X.astype(ml_dtypes.bfloat16)
    for w in range(NW):
        lo = w * WIN
        hi = min(lo + WIN, N_NODES)
        if hi > lo:
            xt[w * WROWS:w * WROWS + (hi - lo)] = Xb[lo:hi]
    return xt


def kernel(X, weights, row_pointers, column_index, blockPartition,
           edgeToColumn, edgeToRow):
    from concourse.bass_utils import run_bass_kernel_spmd

    X = np.asarray(X, dtype=np.float32)
    W = np.ascontiguousarray(np.asarray(weights), dtype=np.float32)
    edges, w_pad = _edge_matrix(row_pointers, column_index)
    orders, m_widths, counts = _plan(edges)
    xt = _make_xt(X)

    nc = _get_nc(m_widths)
    in_maps = []
    for c in range(N_CORES):
        in_maps.append({
            "XT": xt,
            "W": W,
            "idx": _build_idx(edges, orders, m_widths, c),
        })
    last_exc = None
    for _attempt in range(3):
        try:
            res = run_bass_kernel_spmd(nc, in_maps,
                                       core_ids=list(range(N_CORES)))
            break
        except Exception as exc:  # transient NRT/axon errors recover on retry
            last_exc = exc
            time.sleep(15)
    else:
        raise last_exc
    out = np.empty((N_NODES, D), np.float32)
    for c in range(N_CORES):
        rows = res.results[c]["out"]
        out[orders[c, :NODES_PER_CORE]] = rows
    return out
